# revision 32
# baseline (speedup 1.0000x reference)
"""Trainium2 Bass kernel for nn_EncodingInputLayer (embedding_lookup).

Math background
---------------
The reference computes, per batch b:
    v   = one_hot(x[:, :20], 10).reshape(B, 200) @ fc_w.T + fc_b      (B, 9)
    v_map  = broadcast_to(v,      (48, 48, B, 9)).reshape(B, 9, 48, 48)
    o_map  = broadcast_to(others, (48, 48, B, 23)).reshape(B, 23, 48, 48)
    out = all_w conv1x1( concat(oh_w conv1x1 v_map + oh_b,
                                ot_w conv1x1 o_map + ot_b) ) + all_b

The broadcast+raw-reshape *scrambles* batches; working the indexing
through shows batch b's output depends only on b mod 8:

    out[b, c, 9*beta + eps] = Map[b % 8]
    Map[m][c, 9 beta + eps] = sum_e  A1[c, e] v[256((m+e)%8) + beta, eps]
                            + sum_j  A2[c, j] o_flat[(5888 m + 2304 j
                                                      + 9 beta + eps) % 47104]
                            + const[c] + rowsum(A1)[c] fc_b[eps]
    A1 = all_w[:, :9] @ oh_w,  A2 = all_w[:, 9:] @ ot_w
    const = all_w[:, :9] @ oh_b + all_w[:, 9:] @ ot_b + all_b

Sharding: pure data parallel over the 8 distinct residues.  Core k gets
inputs rolled by -256k batches so every core runs the identical program
computing its own Map.  Only the unique 32x2304 map is written per
core; the host replicates it to the 256 batches of each residue.

Device layout choices (host pre-packs all O(params)/layout-only data):
 - columns are produced in sigma order  s' = 256 eps + beta  (the host
   applies the inverse permutation), which makes the v-window rhs a
   single fully-regular SBUF->SBUF DMA from vT
 - one-hot masks: x index rows are pre-transposed/replicated on host
   into (c, f)-major tiles so one is_equal per 512-batch group per tile
   builds the masks, and the embedding is 8 matmuls of contract 120/80
 - the final map matmul contracts all 34 rhs rows (9 v-windows, 23
   others-windows, ones row for const, fc_b row) in ONE matmul per
   512-column chunk
"""

import numpy as np
from contextlib import ExitStack

import ml_dtypes
import concourse.bass as bass
import concourse.mybir as mybir
import concourse.tile as tile
from concourse import bacc
from concourse.bass_utils import run_bass_kernel_spmd

F32 = mybir.dt.float32
BF16 = mybir.dt.bfloat16
FP8 = mybir.dt.float8e4
NPBF16 = ml_dtypes.bfloat16
NPFP8 = ml_dtypes.float8_e4m3

B = 2048
NF = 43           # flat features per batch
N1 = 20           # one-hot index features
NO = 23           # passthrough features
NCLS = 10         # classes per one-hot
EMB = 9
OUTC = 32
H = W = 48
S = H * W         # 2304
NCORES = 8
BPC = B // NCORES  # 256 output batches per core
OLEN = B * NO      # 47104
NA = 6 * N1        # 120 rows: classes 0..5
NB = 4 * N1        # 80 rows: classes 6..9
G = 4              # 512-batch groups
GW = B // G        # 512


def _emit(nc: bass.Bass):
    xrepA = nc.dram_tensor("xrepA", [NA, B], FP8, kind="ExternalInput").ap()
    xrepB = nc.dram_tensor("xrepB", [NB, B], FP8, kind="ExternalInput").ap()
    tabsA = nc.dram_tensor("tabsA", [NA, EMB + 1], BF16, kind="ExternalInput").ap()
    tabsB = nc.dram_tensor("tabsB", [NB, EMB + 1], BF16, kind="ExternalInput").ap()
    rhs_c = nc.dram_tensor("rhs_c", [NO + 2, S], BF16, kind="ExternalInput").ap()
    lhsT33 = nc.dram_tensor("lhsT33", [8 + NO + 2, OUTC], BF16,
                            kind="ExternalInput").ap()
    out = nc.dram_tensor("out", [OUTC, S], F32, kind="ExternalOutput").ap()

    with ExitStack() as ctx:
        tc = ctx.enter_context(tile.TileContext(nc))
        consts = ctx.enter_context(tc.tile_pool(name="consts", bufs=1))
        psum_v = ctx.enter_context(tc.tile_pool(name="psum_v", bufs=G, space="PSUM"))
        psum_m = ctx.enter_context(tc.tile_pool(name="psum_m", bufs=3, space="PSUM"))

        # --- loads -------------------------------------------------------
        xA = consts.tile([NA, B], FP8)
        xB = consts.tile([NB, B], FP8)
        tA = consts.tile([NA, EMB + 1], BF16)
        tB = consts.tile([NB, EMB + 1], BF16)
        rhs = consts.tile([8 + NO + 2, S], BF16)
        lt = consts.tile([8 + NO + 2, OUTC], BF16)
        # xA column-chunked per 512-batch group across 4 queues so mask g0
        # starts as soon as its own chunk lands; small tables lead the
        # queues that feed the first matmuls.
        nc.sync.dma_start(xA[:, 0:GW], xrepA[:, 0:GW])
        nc.scalar.dma_start(xA[:, GW:2 * GW], xrepA[:, GW:2 * GW])
        nc.gpsimd.dma_start(tA, tabsA)
        nc.sync.dma_start(xA[:, 2 * GW:3 * GW], xrepA[:, 2 * GW:3 * GW])
        nc.scalar.dma_start(xA[:, 3 * GW:B], xrepA[:, 3 * GW:B])
        nc.gpsimd.dma_start(tB, tabsB)
        nc.sync.dma_start(xB[:, 0:B // 2], xrepB[:, 0:B // 2])
        nc.scalar.dma_start(xB[:, B // 2:B], xrepB[:, B // 2:B])
        nc.sync.dma_start(rhs[8:8 + NO + 2, :], rhs_c)
        nc.gpsimd.dma_start(lt, lhsT33)

        # --- one-hot masks + embedding matmul ----------------------------
        # xrep rows hold x[b, f] - c, so mask[(c, f), b] = (x[b, f] == c)
        # is a single is_equal against the immediate 0.0 per slice.
        mA = consts.tile([NA, B], BF16)
        mB = consts.tile([NB, B], BF16)
        for g in range(G):
            sl = slice(GW * g, GW * (g + 1))
            nc.vector.tensor_scalar(
                out=mA[:, sl], in0=xA[:, sl], scalar1=0.0,
                scalar2=None, op0=mybir.AluOpType.is_equal,
            )
        for g in range(G):
            sl = slice(GW * g, GW * (g + 1))
            nc.vector.tensor_scalar(
                out=mB[:, sl], in0=xB[:, sl], scalar1=0.0,
                scalar2=None, op0=mybir.AluOpType.is_equal,
            )

        # Per group: A-part (contract 120) + B-part (contract 80, stop) in
        # that group's own psum tile, copy to vT, then shuffle the two
        # finished v-window rows straight into the map rhs via SBUF->SBUF
        # DMA: rhs[e, 256 eps + beta] = vT[eps, 256 e + beta].  Each DMA has
        # a single-partition dst so neither AP moves its partition axis.
        # (The e = 8 wrap row is folded into lhsT row 0 on the host.)
        vT = consts.tile([EMB, B], BF16)
        copy = mybir.ActivationFunctionType.Copy
        for g in range(G):
            sl = slice(GW * g, GW * (g + 1))
            pv = psum_v.tile([EMB, GW], F32, tag="v")
            nc.tensor.matmul(pv, lhsT=tA[:, 0:EMB], rhs=mA[:, sl],
                             start=True, stop=False)
            nc.tensor.matmul(pv, lhsT=tB[:, 0:EMB], rhs=mB[:, sl],
                             start=False, stop=True)
            if g == G - 1:
                nc.scalar.activation(vT[:, GW * g:GW * g + BPC], pv[:, 0:BPC],
                                     copy)
                nc.vector.tensor_copy(vT[:, GW * g + BPC:GW * (g + 1)],
                                      pv[:, BPC:GW])
            elif g % 2 == 0:
                nc.scalar.activation(vT[:, sl], pv, copy)
            else:
                nc.vector.tensor_copy(vT[:, sl], pv)
            for e in (2 * g, 2 * g + 1):
                eng = nc.sync if e % 2 == 0 else nc.gpsimd
                eng.dma_start(rhs[e:e + 1, :], vT[:, BPC * e:BPC * (e + 1)])

        # --- map matmul + streamed output --------------------------------
        # (34, 32).T @ (34, 2304) -> psum (32, 2304) in 512-col chunks,
        # staged through SBUF (DMA cannot read PSUM) and streamed out.
        map_sb = consts.tile([OUTC, S], F32)
        for ch in range(5):
            sz = 512 if ch < 4 else 256
            sl = slice(512 * ch, 512 * ch + sz)
            pm = psum_m.tile([OUTC, 512], F32, tag="m")
            nc.tensor.matmul(pm[:, 0:sz], lhsT=lt, rhs=rhs[:, sl],
                             start=True, stop=True)
            if ch < 4:
                if ch % 2 == 0:
                    nc.vector.tensor_copy(map_sb[:, sl], pm[:, 0:sz])
                else:
                    nc.scalar.activation(map_sb[:, sl], pm[:, 0:sz], copy)
                eng = nc.sync if ch % 2 == 0 else nc.scalar
                eng.dma_start(out[:, sl], map_sb[:, sl])
            else:
                # last chunk: halves on both engines/queues to shorten the
                # completion tail
                h0 = slice(512 * ch, 512 * ch + sz // 2)
                h1 = slice(512 * ch + sz // 2, 512 * ch + sz)
                nc.vector.tensor_copy(map_sb[:, h0], pm[:, 0:sz // 2])
                nc.scalar.activation(map_sb[:, h1], pm[:, sz // 2:sz], copy)
                nc.sync.dma_start(out[:, h0], map_sb[:, h0])
                nc.scalar.dma_start(out[:, h1], map_sb[:, h1])

    return nc


_NC_CACHE: dict = {}


def _get_nc():
    if "nc" not in _NC_CACHE:
        nc = bacc.Bacc("TRN2", target_bir_lowering=False, debug=False,
                       num_devices=NCORES)
        _emit(nc)
        nc.compile()
        _NC_CACHE["nc"] = nc
    return _NC_CACHE["nc"]


def _prep_inputs(x, fc_w, fc_b, oh_w, oh_b, ot_w, ot_b, all_w, all_b):
    xf = np.asarray(x, dtype=np.float32).reshape(B, NF)
    fc_w = np.asarray(fc_w, dtype=np.float32)
    fc_b = np.asarray(fc_b, dtype=np.float32)

    # folded channel-mixing weights (tiny, batch-independent)
    A1 = np.asarray(all_w, np.float32)[:, :EMB] @ np.asarray(oh_w, np.float32)
    A2 = np.asarray(all_w, np.float32)[:, EMB:] @ np.asarray(ot_w, np.float32)
    const = (np.asarray(all_w, np.float32)[:, :EMB] @ np.asarray(oh_b, np.float32)
             + np.asarray(all_w, np.float32)[:, EMB:] @ np.asarray(ot_b, np.float32)
             + np.asarray(all_b, np.float32))
    # wrap fold: the e = 8 v-window equals the e = 0 window, so its weight
    # column collapses onto e = 0 and the rhs carries only 8 v rows.
    A1f = A1[:, 0:8].copy()
    A1f[:, 0] += A1[:, 8]
    lhsT33 = np.concatenate(
        [A1f.T, A2.T, const[None, :], A1.sum(1)[None, :]], axis=0
    ).astype(NPBF16)

    # fc_w tables in (c, f)-row order + the per-row class id column
    arr = fc_w.reshape(EMB, N1, NCLS).transpose(2, 1, 0)    # [c, f, e]
    tabsA = np.concatenate(
        [arr[0:6].reshape(NA, EMB),
         np.repeat(np.arange(6, dtype=np.float32), N1)[:, None]], axis=1
    ).astype(NPBF16)
    tabsB = np.concatenate(
        [arr[6:10].reshape(NB, EMB),
         np.repeat(np.arange(6, 10, dtype=np.float32), N1)[:, None]], axis=1
    ).astype(NPBF16)
    csubA = np.repeat(np.arange(6, dtype=np.float32), N1)[:, None]
    csubB = np.repeat(np.arange(6, 10, dtype=np.float32), N1)[:, None]

    jj = np.arange(NO)[:, None, None]
    ee = np.arange(EMB)[None, :, None]
    bb = np.arange(BPC)[None, None, :]
    w_idx = (2304 * jj + 9 * bb + ee) % OLEN                # (23, 9, 256)
    fcb_row = np.repeat(fc_b, BPC)[None, :]
    ones_row = np.ones((1, S), dtype=np.float32)

    in_maps = []
    for k in range(NCORES):
        idx_k = np.roll(xf[:, :N1], -BPC * k, axis=0)       # (2048, 20)
        of_k = np.roll(np.ascontiguousarray(xf[:, N1:]).reshape(-1), -NO * BPC * k)
        xid_t = np.ascontiguousarray(idx_k.T)               # (20, 2048)
        xrepA = (np.tile(xid_t, (6, 1)) - csubA).astype(NPFP8)
        xrepB = (np.tile(xid_t, (4, 1)) - csubB).astype(NPFP8)
        rhs_c = np.concatenate(
            [of_k[w_idx].reshape(NO, S), ones_row, fcb_row], axis=0
        ).astype(NPBF16)
        in_maps.append({
            "xrepA": np.ascontiguousarray(xrepA),
            "xrepB": np.ascontiguousarray(xrepB),
            "tabsA": np.ascontiguousarray(tabsA),
            "tabsB": np.ascontiguousarray(tabsB),
            "rhs_c": np.ascontiguousarray(rhs_c),
            "lhsT33": np.ascontiguousarray(lhsT33),
        })
    return in_maps


def kernel(x, fc_w, fc_b, oh_w, oh_b, ot_w, ot_b, all_w, all_b):
    nc = _get_nc()
    in_maps = _prep_inputs(x, fc_w, fc_b, oh_w, oh_b, ot_w, ot_b, all_w, all_b)
    res = run_bass_kernel_spmd(nc, in_maps, list(range(NCORES)))
    full = np.empty((B, OUTC, H, W), dtype=np.float32)
    for k in range(NCORES):
        md = res.results[k]["out"]                          # (32, 2304) sigma order
        m2 = md.reshape(OUTC, EMB, BPC).transpose(0, 2, 1).reshape(OUTC, H, W)
        full[k::NCORES] = m2[None]
    return full
